# revision 23
# baseline (speedup 1.0000x reference)
"""Trainium2 Bass kernel for 2-layer GAT (nn_GAT_50603304681766).

Node-parallel across 8 cores. Per core:
  phase A (For_i): t1 = x_shard @ [W1 | W1@Asrc | W1@Adst] (f16 PE) ->
    f32 table rows [h|s] -> AllGather T1; d-values kept local (bf16 hi/lo).
  phase B (unrolled): indirect-DMA gather T1[src] rows -> DRAM staging
    (indirect DMA cannot live inside a hardware loop).
  phase C (For_i): per dst-tile: load staged rows, build one-hot scatter
    matrices on device (iota+is_equal), d-expand via matmul,
    g = exp(leakyrelu(s+d)), weighted scatter matmul (messages+denoms),
    normalize, +bias, ELU, then fused layer-2 linear: PE-transpose h2 and
    matmul with W2a -> T2 rows + d2 -> AllGather T2.
  phase D/E: same gather + message pass for layer 2 -> log_softmax -> out.
Only f16 activations + int indices ship to the device; Bass build and
NEFF compile run in a background thread overlapped with the transfer.
"""
import threading
import numpy as np

N = 50000
E0 = 800000
F_IN = 256
H = 4
C1 = 64
C2 = 32
NEG = 0.2
NC = 8
NSH = 6250            # dst nodes per core
NSHP = 6272           # padded to 49*128
NT = 49               # dst tiles per core
NBLK = 19             # edge blocks (of 128) per dst tile
EB = NBLK * 128       # edge slots per dst tile
ROWS = NC * NSHP      # allgathered table rows = 50176
RW1 = 260             # T1 row: h(256) + s(4)  [f32]
RW2 = 132             # T2 row: h2'(128) + s2(4) [f32]


def _host_prep(x, edge_index, W1, as1, ad1, b1, W2, as2, ad2, b2):
    ei = np.asarray(edge_index).astype(np.int64)
    loop = np.arange(N, dtype=np.int64)
    src = np.concatenate([ei[0], loop])
    dst = np.concatenate([ei[1], loop])
    Etot = src.shape[0]

    def aug(W, a_s, a_d, heads, ch):
        S = np.zeros((heads * ch, heads), np.float32)
        D = np.zeros((heads * ch, heads), np.float32)
        for h in range(heads):
            S[h * ch:(h + 1) * ch, h] = a_s[h]
            D[h * ch:(h + 1) * ch, h] = a_d[h]
        return np.concatenate([W, W @ S, W @ D], axis=1)  # [fin, hc+2h]

    W1a = aug(np.asarray(W1, np.float32), np.asarray(as1), np.asarray(ad1),
              H, C1).astype(np.float16)   # [256, 264]
    W2a = aug(np.asarray(W2, np.float32), np.asarray(as2), np.asarray(ad2),
              H, C2).astype(np.float16)   # [256, 136]

    core_of = dst // NSH
    loc = dst - core_of * NSH
    tile_of = loc >> 7
    dloc = loc & 127
    srow = ((src // NSH) * NSHP + (src % NSH)).astype(np.int32)

    key = (core_of * NT + tile_of).astype(np.int64)
    order = np.argsort(key, kind="stable")
    ks = key[order]
    srow_s = srow[order]
    dl_s = dloc[order].astype(np.int16)
    counts = np.bincount(ks, minlength=NC * NT)
    assert counts.max() <= EB, f"tile overflow {counts.max()}"
    starts = np.zeros(NC * NT, np.int64)
    np.cumsum(counts[:-1], out=starts[1:])
    pos = np.arange(Etot, dtype=np.int64) - starts[ks]
    slot = ks * EB + pos
    idx_flat = np.zeros(NC * NT * EB, np.int32)
    dl_flat = np.full(NC * NT * EB, 255, np.int16)
    idx_flat[slot] = srow_s
    dl_flat[slot] = dl_s
    # slot linear order within a tile is (b, e): block-major
    idx_t = np.ascontiguousarray(
        idx_flat.reshape(NC, NT, NBLK, 128).transpose(0, 1, 3, 2))
    dlc = np.ascontiguousarray(
        dl_flat.reshape(NC, NT, NBLK, 128).transpose(0, 1, 3, 2))
    dlr = dl_flat.reshape(NC, NT, EB)

    xf = np.asarray(x, np.float32).astype(np.float16)
    xs = np.zeros((NC, F_IN, NSHP), np.float16)
    xs[:, :, :NSH] = xf.reshape(NC, NSH, F_IN).transpose(0, 2, 1)

    b1r = np.asarray(b1, np.float32).reshape(1, H * C1)
    b2r = np.asarray(b2, np.float32).reshape(1, H * C2)
    return W1a, W2a, idx_t, dlc, dlr, xs, b1r, b2r


def _build_nc():
    import concourse.bass as bass
    import concourse.tile as tile
    from concourse import mybir
    from concourse.bass import IndirectOffsetOnAxis, ds, ts

    f32 = mybir.dt.float32
    f16 = mybir.dt.float16
    bf16 = mybir.dt.bfloat16
    i32 = mybir.dt.int32
    i16 = mybir.dt.int16
    AF = mybir.ActivationFunctionType
    ALU = mybir.AluOpType

    nc = bass.Bass()
    xT = nc.declare_dram_parameter("xT", [F_IN, NSHP], f16, isOutput=False)
    w1 = nc.declare_dram_parameter("w1", [F_IN, RW1 + 4], f16, isOutput=False)
    w2 = nc.declare_dram_parameter("w2", [F_IN, RW2 + 4], f16, isOutput=False)
    idxp = nc.declare_dram_parameter("idx", [NT, 128, NBLK], i32, isOutput=False)
    dlcp = nc.declare_dram_parameter("dlc", [NT, 128, NBLK], i16, isOutput=False)
    dlrp = nc.declare_dram_parameter("dlr", [NT, EB], i16, isOutput=False)
    b1p = nc.declare_dram_parameter("b1r", [1, H * C1], f32, isOutput=False)
    b2p = nc.declare_dram_parameter("b2r", [1, H * C2], f32, isOutput=False)
    outp = nc.declare_dram_parameter("out", [NT, 128, H * C2], f16, isOutput=True)

    t1_loc = nc.dram_tensor("t1_loc", [NSHP, RW1], f32)
    d1_loc = nc.dram_tensor("d1_loc", [NSHP, 8], bf16)
    t2_loc = nc.dram_tensor("t2_loc", [NSHP, RW2], f32)
    d2_loc = nc.dram_tensor("d2_loc", [NSHP, 8], bf16)
    T1 = nc.dram_tensor("T1ag", [ROWS, RW1], f32, addr_space="Shared")
    T2 = nc.dram_tensor("T2ag", [ROWS, RW2], f32, addr_space="Shared")
    # one staging buffer, reused by both layers (L2 rows are narrower)
    v_dram = nc.dram_tensor("vst", [NT, 128, NBLK * RW1], f32)

    def pack_d(ap, acc, rw, d_dram, t, tagsfx):
        """acc[:, rw:rw+4] f32 -> bf16 hi/lo row, store to d_dram[t*128...]."""
        drow = ap.tile([128, 8], bf16, tag="drow" + tagsfx)
        d_hi32 = ap.tile([128, 4], f32, tag="dhi" + tagsfx)
        nc.vector.tensor_copy(drow[:, 0:4], acc[:, rw:rw + 4])
        nc.vector.tensor_copy(d_hi32[:], drow[:, 0:4])
        d_lo = ap.tile([128, 4], f32, tag="dlo" + tagsfx)
        nc.vector.tensor_tensor(out=d_lo[:], in0=acc[:, rw:rw + 4],
                                in1=d_hi32[:], op=ALU.subtract)
        nc.vector.tensor_copy(drow[:, 4:8], d_lo[:])
        nc.sync.dma_start(d_dram[ts(t, 128), :], drow[:])

    # ---------- phase A: t1 = xT.T @ W1a (For_i) ----------
    with tile.TileContext(nc) as tc:
        with (
            tc.tile_pool(name="aw", bufs=1) as wp,
            tc.tile_pool(name="aa", bufs=1) as ap,
            tc.tile_pool(name="aps", bufs=1, space="PSUM") as pp,
        ):
            w1_t = wp.tile([128, 2, RW1 + 4], f16)
            nc.sync.dma_start(
                w1_t[:], w1[:, :].rearrange("(k p) c -> p k c", p=128))
            with tc.For_i(0, NT) as t:
                xt = ap.tile([128, 2, 128], f16, tag="xt")
                nc.sync.dma_start(
                    xt[:],
                    xT[:, ds(t * 128, 128)].rearrange("(k p) c -> p k c", p=128))
                acc = pp.tile([128, RW1 + 4], f32, tag="acc")
                nc.tensor.matmul(out=acc[:], lhsT=xt[:, 0, :],
                                 rhs=w1_t[:, 0, :], start=True, stop=False)
                nc.tensor.matmul(out=acc[:], lhsT=xt[:, 1, :],
                                 rhs=w1_t[:, 1, :], start=False, stop=True)
                row = ap.tile([128, RW1], f32, tag="row")
                nc.vector.tensor_copy(row[:], acc[:, 0:RW1])
                nc.sync.dma_start(t1_loc[ts(t, 128), :], row[:])
                pack_d(ap, acc, RW1, d1_loc, t, "a")

    with nc.semaphore("cc1") as cc1:
        nc.gpsimd.collective_compute(
            "AllGather", mybir.AluOpType.bypass,
            replica_groups=[list(range(NC))],
            ins=[t1_loc[:, :].opt()], outs=[T1[:, :].opt()],
        ).then_inc(cc1, 1)
        nc.gpsimd.wait_ge(cc1, 1)

    # ---------- gather pre-pass (unrolled; indirect DMA can't be looped) ----
    def gather_pass(Tag, rw):
        with tile.TileContext(nc) as tc:
            with tc.tile_pool(name="g", bufs=2) as gp:
                for t in range(NT):
                    idx_t = gp.tile([128, NBLK], i32, tag="idx")
                    nc.sync.dma_start(idx_t[:], idxp[t, :, :])
                    vg = gp.tile([128, NBLK, rw], f32, tag="vg")
                    for b in range(NBLK):
                        nc.gpsimd.indirect_dma_start(
                            out=vg[:, b, :], out_offset=None, in_=Tag[:, :],
                            in_offset=IndirectOffsetOnAxis(
                                ap=idx_t[:, b:b + 1], axis=0))
                    nc.sync.dma_start(
                        v_dram[t, :, 0:NBLK * rw],
                        vg[:].rearrange("p b c -> p (b c)"))

    gather_pass(T1, RW1)

    # ---------- phase C: L1 message pass + fused L2 linear (For_i) ----------
    def msg_loop(tc, d_loc_t, rw, hw, hoist_cb, out_cb):
        hwq = hw // 4
        with (
            tc.tile_pool(name="mc", bufs=1) as cp,
            tc.tile_pool(name="ma", bufs=1) as ap,
            tc.tile_pool(name="mps", bufs=1, space="PSUM") as pp,
        ):
            iof = cp.tile([128, NBLK, 128], i16)
            nc.gpsimd.iota(iof[:], pattern=[[0, NBLK], [1, 128]], base=0,
                           channel_multiplier=0)
            pidx = cp.tile([128, 1], i16)
            nc.gpsimd.iota(pidx[:], pattern=[[0, 1]], base=0,
                           channel_multiplier=1)
            hoist = hoist_cb(cp)
            with tc.For_i(0, NT) as t:
                v = ap.tile([128, NBLK, rw], f32, tag="v")
                nc.sync.dma_start(
                    v[:].rearrange("p b c -> p (b c)"),
                    v_dram[ds(t, 1), :, 0:NBLK * rw])
                dtab = ap.tile([128, 8], bf16, tag="dtab")
                nc.sync.dma_start(dtab[:], d_loc_t[ts(t, 128), :])
                dlc_t = ap.tile([128, NBLK], i16, tag="dlc")
                nc.sync.dma_start(dlc_t[:], dlcp[ds(t, 1), :, :])
                m_t = ap.tile([128, NBLK, 128], bf16, tag="m")
                nc.vector.tensor_tensor(
                    out=m_t[:], in0=iof[:],
                    in1=dlc_t[:].unsqueeze(2).to_broadcast([128, NBLK, 128]),
                    op=ALU.is_equal)
                dlr_t = ap.tile([128, EB], i16, tag="dlr")
                nc.sync.dma_start(
                    dlr_t[:], dlrp[ds(t, 1), :].to_broadcast([128, EB]))
                mt_t = ap.tile([128, EB], bf16, tag="mt")
                nc.vector.tensor_tensor(
                    out=mt_t[:], in0=dlr_t[:],
                    in1=pidx[:].to_broadcast([128, EB]), op=ALU.is_equal)
                dex = pp.tile([128, NBLK * 8], f32, tag="dex")
                for b in range(NBLK):
                    nc.tensor.matmul(out=dex[:, b * 8:(b + 1) * 8],
                                     lhsT=mt_t[:, b * 128:(b + 1) * 128],
                                     rhs=dtab[:], start=True, stop=True)
                dsb = ap.tile([128, NBLK, 8], f32, tag="dsb")
                nc.vector.tensor_copy(
                    dsb[:], dex[:].rearrange("p (b k) -> p b k", k=8))
                d32 = ap.tile([128, NBLK, 4], f32, tag="d32")
                nc.vector.tensor_tensor(out=d32[:], in0=dsb[:, :, 0:4],
                                        in1=dsb[:, :, 4:8], op=ALU.add)
                e32 = ap.tile([128, NBLK, 4], f32, tag="e32")
                nc.vector.tensor_tensor(out=e32[:], in0=v[:, :, hw:hw + 4],
                                        in1=d32[:], op=ALU.add)
                e_s = ap.tile([128, NBLK, 4], f32, tag="es")
                nc.vector.tensor_scalar_mul(e_s[:], e32[:], NEG)
                nc.vector.tensor_tensor(out=e32[:], in0=e32[:], in1=e_s[:],
                                        op=ALU.max)
                g = ap.tile([128, NBLK, 4], f32, tag="g")
                nc.scalar.activation(g[:], e32[:], AF.Exp)
                wv = ap.tile([128, NBLK, hw + 4], bf16, tag="wv")
                nc.vector.tensor_tensor(
                    out=wv[:, :, 0:hw].rearrange("p b (h c) -> p b h c", h=4),
                    in0=v[:, :, 0:hw].rearrange("p b (h c) -> p b h c", h=4),
                    in1=g[:].unsqueeze(3).to_broadcast([128, NBLK, 4, hwq]),
                    op=ALU.mult)
                nc.vector.tensor_copy(wv[:, :, hw:hw + 4], g[:])
                acc = pp.tile([128, hw + 4], f32, tag="acc2")
                for b in range(NBLK):
                    nc.tensor.matmul(out=acc[:], lhsT=m_t[:, b, :],
                                     rhs=wv[:, b, :], start=(b == 0),
                                     stop=(b == NBLK - 1))
                out_cb(t, acc, ap, pp, hoist)

    def make_idn(cp):
        idn = cp.tile([128, 128], f32, tag="idn")
        iot = cp.tile([128, 1], i32, tag="iot")
        nc.gpsimd.iota(iot[:], pattern=[[0, 1]], base=0, channel_multiplier=1)
        iotf = cp.tile([128, 1], f32, tag="iotf")
        nc.vector.tensor_copy(iotf[:], iot[:])
        i2 = cp.tile([128, 128], i32, tag="i2")
        nc.gpsimd.iota(i2[:], pattern=[[1, 128]], base=0, channel_multiplier=0)
        eqi = cp.tile([128, 128], f32, tag="eqi")
        nc.vector.tensor_copy(eqi[:], i2[:])
        nc.vector.tensor_tensor(out=idn[:], in0=eqi[:],
                                in1=iotf[:].to_broadcast([128, 128]),
                                op=ALU.is_equal)
        return idn

    with tile.TileContext(nc) as tc:
        def l1_hoist(cp):
            b1_t = cp.tile([128, 256], f32, tag="b1t")
            nc.sync.dma_start(b1_t[:], b1p[0:1, :].to_broadcast([128, 256]))
            w2_t = cp.tile([128, 2, RW2 + 4], f16, tag="w2t")
            nc.sync.dma_start(
                w2_t[:], w2[:, :].rearrange("(k p) c -> p k c", p=128))
            return {"b1": b1_t, "w2": w2_t, "idn": make_idn(cp)}

        def l1_out(t, acc, ap, pp, hoist):
            rec = ap.tile([128, 4], f32, tag="rec")
            nc.vector.reciprocal(rec[:], acc[:, 256:260])
            h2 = ap.tile([128, 256], f32, tag="h2")
            nc.vector.tensor_tensor(
                out=h2[:].rearrange("p (h c) -> p h c", h=4),
                in0=acc[:, 0:256].rearrange("p (h c) -> p h c", h=4),
                in1=rec[:].unsqueeze(2).to_broadcast([128, 4, 64]),
                op=ALU.mult)
            nc.vector.tensor_tensor(out=h2[:], in0=h2[:], in1=hoist["b1"][:],
                                    op=ALU.add)
            # ELU: max(x, exp(min(x,0)) - 1)
            mn = ap.tile([128, 256], f32, tag="mn")
            nc.vector.tensor_scalar_min(mn[:], h2[:], 0.0)
            nc.scalar.activation(mn[:], mn[:], AF.Exp)
            nc.vector.tensor_scalar_add(mn[:], mn[:], -1.0)
            nc.vector.tensor_tensor(out=h2[:], in0=h2[:], in1=mn[:], op=ALU.max)
            # fused L2 linear: transpose h2 chunks, matmul with W2a
            tps = []
            for kk in range(2):
                tp = pp.tile([128, 128], f32, tag=f"tp{kk}")
                nc.tensor.transpose(out=tp[:], in_=h2[:, kk * 128:(kk + 1) * 128],
                                    identity=hoist["idn"][:])
                tk = ap.tile([128, 128], f16, tag=f"tps{kk}")
                nc.vector.tensor_copy(tk[:], tp[:])
                tps.append(tk)
            acc2 = pp.tile([128, RW2 + 4], f32, tag="acc3")
            w2_t = hoist["w2"]
            nc.tensor.matmul(out=acc2[:], lhsT=tps[0][:], rhs=w2_t[:, 0, :],
                             start=True, stop=False)
            nc.tensor.matmul(out=acc2[:], lhsT=tps[1][:], rhs=w2_t[:, 1, :],
                             start=False, stop=True)
            row = ap.tile([128, RW2], f32, tag="row2")
            nc.vector.tensor_copy(row[:], acc2[:, 0:RW2])
            nc.sync.dma_start(t2_loc[ts(t, 128), :], row[:])
            pack_d(ap, acc2, RW2, d2_loc, t, "c")

        msg_loop(tc, d1_loc, RW1, 256, l1_hoist, l1_out)

    with nc.semaphore("cc2") as cc2:
        nc.gpsimd.collective_compute(
            "AllGather", mybir.AluOpType.bypass,
            replica_groups=[list(range(NC))],
            ins=[t2_loc[:, :].opt()], outs=[T2[:, :].opt()],
        ).then_inc(cc2, 1)
        nc.gpsimd.wait_ge(cc2, 1)

    gather_pass(T2, RW2)

    # ---------- phase E: L2 message pass -> log_softmax -> out (For_i) ------
    with tile.TileContext(nc) as tc:
        def l2_hoist(cp):
            b2_t = cp.tile([128, 128], f32, tag="b2t")
            nc.sync.dma_start(b2_t[:], b2p[0:1, :].to_broadcast([128, 128]))
            return {"b2": b2_t}

        def l2_out(t, acc, ap, pp, hoist):
            rec = ap.tile([128, 4], f32, tag="rec2")
            nc.vector.reciprocal(rec[:], acc[:, 128:132])
            o = ap.tile([128, 128], f32, tag="o")
            nc.vector.tensor_tensor(
                out=o[:].rearrange("p (h c) -> p h c", h=4),
                in0=acc[:, 0:128].rearrange("p (h c) -> p h c", h=4),
                in1=rec[:].unsqueeze(2).to_broadcast([128, 4, 32]),
                op=ALU.mult)
            nc.vector.tensor_tensor(out=o[:], in0=o[:], in1=hoist["b2"][:],
                                    op=ALU.add)
            mx = ap.tile([128, 1], f32, tag="mx")
            nc.vector.reduce_max(mx[:], o[:], axis=mybir.AxisListType.X)
            nc.vector.tensor_scalar(out=o[:], in0=o[:], scalar1=mx[:, 0:1],
                                    scalar2=None, op0=ALU.subtract)
            ex = ap.tile([128, 128], f32, tag="ex")
            nc.scalar.activation(ex[:], o[:], AF.Exp)
            sm = ap.tile([128, 1], f32, tag="sm")
            nc.vector.reduce_sum(sm[:], ex[:], axis=mybir.AxisListType.X)
            nc.scalar.activation(sm[:], sm[:], AF.Ln)
            nc.vector.tensor_scalar(out=o[:], in0=o[:], scalar1=sm[:, 0:1],
                                    scalar2=None, op0=ALU.subtract)
            o16 = ap.tile([128, 128], f16, tag="o16")
            nc.vector.tensor_copy(o16[:], o[:])
            nc.sync.dma_start(outp[ds(t, 1), :, :], o16[:])

        msg_loop(tc, d2_loc, RW2, 128, l2_hoist, l2_out)

    return nc


def _split_sync_waits(nc, max_waits=1):
    import concourse.mybir as mybir
    ctr = [0]
    for f in nc.m.functions:
        for blk in f.blocks:
            new_list = []
            for ins in blk.instructions:
                si = ins.sync_info
                waits = list(si.on_wait) if si is not None and si.on_wait else []
                if len(waits) > max_waits:
                    keep = waits[:max_waits]
                    rest = waits[max_waits:]
                    for i in range(0, len(rest), max_waits):
                        ctr[0] += 1
                        nop = mybir.InstNoOp(
                            name=f"I-wsplit-{ctr[0]}", ins=[], outs=[],
                            engine=ins.engine, bass_nofuse=True)
                        nop.sync_info = mybir.SyncInfo(
                            on_wait=rest[i:i + max_waits], on_update=[])
                        new_list.append(nop)
                    ins.sync_info = mybir.SyncInfo(
                        on_wait=keep,
                        on_update=list(si.on_update) if si.on_update else [])
                new_list.append(ins)
            blk.instructions[:] = new_list


# input order must match ExternalInput declaration order in _build_nc
_IN_NAMES = ["xT", "w1", "w2", "idx", "dlc", "dlr", "b1r", "b2r"]
_CACHE = {}
_LOCK = threading.Lock()


def _compile():
    """Build the Bass module and compile the sharded executable. Cached."""
    with _LOCK:
        if "compiled" in _CACHE:
            return _CACHE["compiled"]
        import os
        import jax
        import jax.numpy as jnp
        from jax.sharding import Mesh, PartitionSpec, NamedSharding
        try:
            jax.config.update("jax_compilation_cache_dir", "/tmp/jax_bass_cache")
            jax.config.update("jax_persistent_cache_min_entry_size_bytes", -1)
            jax.config.update("jax_persistent_cache_min_compile_time_secs", 0.0)
        except Exception:
            pass
        import warnings
        with warnings.catch_warnings():
            warnings.simplefilter("ignore")
            from jax.experimental.shard_map import shard_map
        import concourse.bass2jax as b2j
        from concourse import mybir

        import time as _t
        import hashlib
        import inspect
        import pickle
        import zstandard

        key_src = inspect.getsource(_build_nc) + inspect.getsource(
            _split_sync_waits) + "v1"
        key = hashlib.sha256(key_src.encode()).hexdigest()[:16]
        bir_path = f"/tmp/bass_gat_bir_{key}.pkl"
        _ts = _t.time()
        meta = None
        try:
            with open(bir_path, "rb") as fh:
                meta = pickle.load(fh)
        except Exception:
            meta = None
        if meta is None:
            nc_full = _build_nc()
            _split_sync_waits(nc_full, 1)
            partition_name = (nc_full.partition_id_tensor.name
                              if nc_full.partition_id_tensor else None)
            in_specs_m, out_specs_m = [], []
            for alloc in nc_full.m.functions[0].allocations:
                if not isinstance(alloc, mybir.MemoryLocationSet):
                    continue
                name = alloc.memorylocations[0].name
                if alloc.kind == "ExternalInput":
                    if name != partition_name:
                        in_specs_m.append(
                            (name, tuple(alloc.tensor_shape),
                             np.dtype(mybir.dt.np(alloc.dtype)).str))
                elif alloc.kind == "ExternalOutput":
                    out_specs_m.append(
                        (name, tuple(alloc.tensor_shape),
                         np.dtype(mybir.dt.np(alloc.dtype)).str))
            meta = {
                "bir_zst": zstandard.ZstdCompressor().compress(
                    nc_full.to_json_bytes()),
                "arch": nc_full.m.arch,
                "has_collectives": nc_full.has_collectives,
                "partition_name": partition_name,
                "in_specs": in_specs_m,
                "out_specs": out_specs_m,
            }
            try:
                tmp = bir_path + ".tmp"
                with open(tmp, "wb") as fh:
                    pickle.dump(meta, fh)
                os.replace(tmp, bir_path)
            except Exception:
                pass

        class _M:
            pass

        class _NcShim:
            def __init__(self, bir, arch, has_cc):
                self._bir = bir
                self.m = _M()
                self.m.arch = arch
                self.has_collectives = has_cc
                self.target_bir_lowering = False
                self.dbg_addr = None

            def to_json_bytes(self):
                return self._bir

        nc = _NcShim(
            zstandard.ZstdDecompressor().decompress(meta["bir_zst"]),
            meta["arch"], meta["has_collectives"])
        _CACHE["t_build"] = _t.time() - _ts
        b2j.install_neuronx_cc_hook()

        partition_name = meta["partition_name"]
        in_names = [s[0] for s in meta["in_specs"]]
        out_names, out_avals, out_shapes = [], [], []
        for name, shape, dts in meta["out_specs"]:
            dtype = np.dtype(dts).type
            out_names.append(name)
            out_avals.append(jax.core.ShapedArray(shape, dtype))
            out_shapes.append((shape, dtype))
        n_params = len(in_names)
        n_outs = len(out_avals)
        all_names = in_names + out_names
        if partition_name is not None:
            all_names = all_names + [partition_name]
        donate = tuple(range(n_params, n_params + n_outs))

        def _body(*args):
            operands = list(args)
            if partition_name is not None:
                operands.append(b2j.partition_id_tensor())
            outs = b2j._bass_exec_p.bind(
                *operands, out_avals=tuple(out_avals),
                in_names=tuple(all_names), out_names=tuple(out_names),
                lowering_input_output_aliases=(),
                sim_require_finite=True, sim_require_nnan=True, nc=nc)
            return tuple(outs)

        devices = jax.devices()[:NC]
        mesh = Mesh(np.asarray(devices), ("core",))
        sharding = NamedSharding(mesh, PartitionSpec("core"))
        in_specs = (PartitionSpec("core"),) * (n_params + n_outs)
        out_specs = (PartitionSpec("core"),) * n_outs
        jitted = jax.jit(
            shard_map(_body, mesh=mesh, in_specs=in_specs,
                      out_specs=out_specs, check_rep=False),
            donate_argnums=donate, keep_unused=True)

        in_structs = [
            jax.ShapeDtypeStruct((NC * shp[0],) + tuple(shp[1:]),
                                 np.dtype(dts).type, sharding=sharding)
            for (_, shp, dts) in meta["in_specs"]]
        zero_structs = [jax.ShapeDtypeStruct((NC * s[0],) + tuple(s[1:]), dt,
                                             sharding=sharding)
                        for (s, dt) in out_shapes]
        _ts = _t.time()
        def _do_compile():
            lowered = jitted.lower(*in_structs, *zero_structs)
            _CACHE["t_lower"] = _t.time() - _ts
            return lowered.compile()
        try:
            compiled = b2j.fast_dispatch_compile(_do_compile)
        except Exception:
            compiled = jitted.lower(*in_structs, *zero_structs).compile()
        _CACHE["t_compile"] = _t.time() - _ts

        zfuns = [jax.jit(lambda s=s, dt=dt: jnp.zeros((NC * s[0],) + tuple(s[1:]), dt),
                         out_shardings=sharding) for (s, dt) in out_shapes]
        # pre-create the first set of donated output buffers on-device now,
        # off the critical path
        _CACHE["zeros"] = [zf() for zf in zfuns]
        _CACHE["compiled"] = (compiled, in_names, out_names, out_shapes,
                              sharding, zfuns)
        return _CACHE["compiled"]


def _warmup_transfer():
    """Tiny transfer issued ASAP: the terminal's first-transfer-after-teardown
    stall (if any) burns here, overlapped with compile + host prep."""
    try:
        import jax
        from jax.sharding import Mesh, PartitionSpec, NamedSharding
        devices = jax.devices()[:NC]
        mesh = Mesh(np.asarray(devices), ("core",))
        sh = NamedSharding(mesh, PartitionSpec("core"))
        d = jax.device_put(np.zeros((NC, 8), np.float32), sh)
        jax.block_until_ready(d)
    except Exception:
        pass


def _start_precompile():
    if "thread" in _CACHE:
        return _CACHE["thread"]
    wt = threading.Thread(target=_warmup_transfer, daemon=True)
    _CACHE["warmup"] = wt
    wt.start()
    th = threading.Thread(target=lambda: _try_compile(), daemon=True)
    _CACHE["thread"] = th
    th.start()
    return th


def _try_compile():
    try:
        _compile()
    except Exception as e:  # surface later in kernel()
        _CACHE["compile_error"] = e


def kernel(**inputs):
    import os
    import time as _time
    dbg = os.environ.get("KERNEL_DEBUG_TIMING")
    t_all = _time.time()
    th = _start_precompile()

    from concurrent.futures import ThreadPoolExecutor
    if any(not isinstance(v, np.ndarray) for v in inputs.values()):
        # jax-array inputs: fetch to host in parallel (d2h RPCs overlap)
        with ThreadPoolExecutor(max_workers=8) as pool:
            futs = {k: pool.submit(np.asarray, v) for k, v in inputs.items()}
            inputs = {k: f.result() for k, f in futs.items()}
    x = np.asarray(inputs["x"], np.float32)
    ei = np.asarray(inputs["edge_index"])
    W1a, W2a, idx_t, dlc, dlr, xs, b1r, b2r = _host_prep(
        x, ei, inputs["W1"], inputs["att_src1"], inputs["att_dst1"],
        inputs["b1"], inputs["W2"], inputs["att_src2"], inputs["att_dst2"],
        inputs["b2"])
    if dbg:
        print(f"[k] host_prep {_time.time()-t_all:.2f}s", flush=True)

    import jax

    t0 = _time.time()
    per_core = {
        "xT": xs,
        "w1": np.broadcast_to(W1a[None], (NC,) + W1a.shape),
        "w2": np.broadcast_to(W2a[None], (NC,) + W2a.shape),
        "idx": idx_t, "dlc": dlc, "dlr": dlr,
        "b1r": np.broadcast_to(b1r[None], (NC,) + b1r.shape),
        "b2r": np.broadcast_to(b2r[None], (NC,) + b2r.shape),
    }
    # flatten [NC, s0, ...] -> [NC*s0, ...] for shard_map axis-0 sharding
    flat = {k: np.ascontiguousarray(v).reshape((-1,) + v.shape[2:])
            for k, v in per_core.items()}

    # ship inputs while the compile thread works
    from jax.sharding import Mesh, PartitionSpec, NamedSharding
    devices = jax.devices()[:NC]
    mesh = Mesh(np.asarray(devices), ("core",))
    sharding = NamedSharding(mesh, PartitionSpec("core"))
    dev_map = {name: jax.device_put(flat[name], sharding) for name in _IN_NAMES}
    if dbg:
        print(f"[k] device_put dispatch {_time.time()-t0:.2f}s", flush=True)

    th.join()
    if "compile_error" in _CACHE:
        raise _CACHE["compile_error"]
    compiled, in_names, out_names, out_shapes, _, zfuns = _CACHE["compiled"]
    if dbg:
        print(f"[k] join {_time.time()-t0:.2f}s (build {_CACHE.get('t_build',0):.2f}"
              f" lower {_CACHE.get('t_lower',0):.2f}"
              f" compile {_CACHE.get('t_compile',0):.2f})", flush=True)
    dev_in = [dev_map[name] for name in in_names]
    zeros = _CACHE.pop("zeros", None) or [zf() for zf in zfuns]
    jax.block_until_ready(dev_in)
    if dbg:
        print(f"[k] inputs ready {_time.time()-t0:.2f}s", flush=True)

    out_arrs = compiled(*dev_in, *zeros)
    jax.block_until_ready(out_arrs)
    if dbg:
        print(f"[k] exec {_time.time()-t0:.2f}s", flush=True)

    oi = out_names.index("out")
    shards = out_arrs[oi].addressable_shards
    full = np.empty((NC * NT, 128, H * C2), np.float16)
    with ThreadPoolExecutor(max_workers=8) as pool:
        def _pull(s):
            full[s.index[0]] = np.asarray(s.data)
        list(pool.map(_pull, shards))
    full = full.reshape(NC, NSHP, H * C2)
    res = full[:, :NSH, :].reshape(N, H * C2).astype(np.float32)
    if dbg:
        print(f"[k] fetch {_time.time()-t0:.2f}s", flush=True)
    kernel.last_wall_s = _time.time() - t0
    kernel.total_wall_s = _time.time() - t_all
    return res


def _graceful_shutdown():
    """Release device state before exit: an abrupt client disconnect leaves
    the axon terminal cleaning up lazily, which stalls the next process's
    first transfer by tens of seconds."""
    try:
        import sys
        if "jax" not in sys.modules:
            return
        import gc
        import jax
        _CACHE.clear()
        gc.collect()
        try:
            jax.clear_caches()
        except Exception:
            pass
        try:
            import jax.extend.backend as jeb
            jeb.clear_backends()
        except Exception:
            pass
        gc.collect()
    except Exception:
        pass


import atexit  # noqa: E402
atexit.register(_graceful_shutdown)

_start_precompile()


# revision 26
# speedup vs baseline: 1.3286x; 1.3286x over previous
"""Trainium2 Bass kernel for 2-layer GAT (nn_GAT_50603304681766).

Node-parallel across 8 cores. Per core:
  phase A (For_i): t1 = x_shard @ [W1 | W1@Asrc | W1@Adst] (f16 PE) ->
    f32 table rows [h|s] -> AllGather T1; d-values kept local (bf16 hi/lo).
  phase B (unrolled): indirect-DMA gather T1[src] rows -> DRAM staging
    (indirect DMA cannot live inside a hardware loop).
  phase C (For_i): per dst-tile: load staged rows, build one-hot scatter
    matrices on device (iota+is_equal), d-expand via matmul,
    g = exp(leakyrelu(s+d)), weighted scatter matmul (messages+denoms),
    normalize, +bias, ELU, then fused layer-2 linear: PE-transpose h2 and
    matmul with W2a -> T2 rows + d2 -> AllGather T2.
  phase D/E: same gather + message pass for layer 2 -> log_softmax -> out.
Only f16 activations + int indices ship to the device; Bass build and
NEFF compile run in a background thread overlapped with the transfer.
"""
import threading
import numpy as np

N = 50000
E0 = 800000
F_IN = 256
H = 4
C1 = 64
C2 = 32
NEG = 0.2
NC = 8
NSH = 6250            # dst nodes per core
NSHP = 6272           # padded to 49*128
NT = 49               # dst tiles per core
NBLK = 19             # edge blocks (of 128) per dst tile
EB = NBLK * 128       # edge slots per dst tile
ROWS = NC * NSHP      # allgathered table rows = 50176
RW1 = 260             # T1 row: h(256) + s(4)  [f32]
RW2 = 132             # T2 row: h2'(128) + s2(4) [f32]


def _prep_x(x):
    xf = np.asarray(x, np.float32).astype(np.float16)
    xs = np.zeros((NC, F_IN, NSHP), np.float16)
    xs[:, :, :NSH] = xf.reshape(NC, NSH, F_IN).transpose(0, 2, 1)
    return xs


def _prep_rest(edge_index, W1, as1, ad1, b1, W2, as2, ad2, b2):
    ei = np.asarray(edge_index).astype(np.int64)
    loop = np.arange(N, dtype=np.int64)
    src = np.concatenate([ei[0], loop])
    dst = np.concatenate([ei[1], loop])
    Etot = src.shape[0]

    def aug(W, a_s, a_d, heads, ch):
        S = np.zeros((heads * ch, heads), np.float32)
        D = np.zeros((heads * ch, heads), np.float32)
        for h in range(heads):
            S[h * ch:(h + 1) * ch, h] = a_s[h]
            D[h * ch:(h + 1) * ch, h] = a_d[h]
        return np.concatenate([W, W @ S, W @ D], axis=1)  # [fin, hc+2h]

    W1a = aug(np.asarray(W1, np.float32), np.asarray(as1), np.asarray(ad1),
              H, C1).astype(np.float16)   # [256, 264]
    W2a = aug(np.asarray(W2, np.float32), np.asarray(as2), np.asarray(ad2),
              H, C2).astype(np.float16)   # [256, 136]

    core_of = dst // NSH
    loc = dst - core_of * NSH
    tile_of = loc >> 7
    dloc = loc & 127
    srow = ((src // NSH) * NSHP + (src % NSH)).astype(np.int32)

    key = (core_of * NT + tile_of).astype(np.int64)
    order = np.argsort(key, kind="stable")
    ks = key[order]
    srow_s = srow[order]
    dl_s = dloc[order].astype(np.int16)
    counts = np.bincount(ks, minlength=NC * NT)
    assert counts.max() <= EB, f"tile overflow {counts.max()}"
    starts = np.zeros(NC * NT, np.int64)
    np.cumsum(counts[:-1], out=starts[1:])
    pos = np.arange(Etot, dtype=np.int64) - starts[ks]
    slot = ks * EB + pos
    idx_flat = np.zeros(NC * NT * EB, np.int32)
    dl_flat = np.full(NC * NT * EB, 255, np.int16)
    idx_flat[slot] = srow_s
    dl_flat[slot] = dl_s
    # slot linear order within a tile is (b, e): block-major
    idx_t = np.ascontiguousarray(
        idx_flat.reshape(NC, NT, NBLK, 128).transpose(0, 1, 3, 2))
    dlc = np.ascontiguousarray(
        dl_flat.reshape(NC, NT, NBLK, 128).transpose(0, 1, 3, 2))
    dlr = dl_flat.reshape(NC, NT, EB)

    b1r = np.asarray(b1, np.float32).reshape(1, H * C1)
    b2r = np.asarray(b2, np.float32).reshape(1, H * C2)
    return W1a, W2a, idx_t, dlc, dlr, b1r, b2r


def _host_prep(x, edge_index, W1, as1, ad1, b1, W2, as2, ad2, b2):
    W1a, W2a, idx_t, dlc, dlr, b1r, b2r = _prep_rest(
        edge_index, W1, as1, ad1, b1, W2, as2, ad2, b2)
    return W1a, W2a, idx_t, dlc, dlr, _prep_x(x), b1r, b2r


def _build_nc():
    import concourse.bass as bass
    import concourse.tile as tile
    from concourse import mybir
    from concourse.bass import IndirectOffsetOnAxis, ds, ts

    f32 = mybir.dt.float32
    f16 = mybir.dt.float16
    bf16 = mybir.dt.bfloat16
    i32 = mybir.dt.int32
    i16 = mybir.dt.int16
    AF = mybir.ActivationFunctionType
    ALU = mybir.AluOpType

    nc = bass.Bass()
    xT = nc.declare_dram_parameter("xT", [F_IN, NSHP], f16, isOutput=False)
    w1 = nc.declare_dram_parameter("w1", [F_IN, RW1 + 4], f16, isOutput=False)
    w2 = nc.declare_dram_parameter("w2", [F_IN, RW2 + 4], f16, isOutput=False)
    idxp = nc.declare_dram_parameter("idx", [NT, 128, NBLK], i32, isOutput=False)
    dlcp = nc.declare_dram_parameter("dlc", [NT, 128, NBLK], i16, isOutput=False)
    dlrp = nc.declare_dram_parameter("dlr", [NT, EB], i16, isOutput=False)
    b1p = nc.declare_dram_parameter("b1r", [1, H * C1], f32, isOutput=False)
    b2p = nc.declare_dram_parameter("b2r", [1, H * C2], f32, isOutput=False)
    outp = nc.declare_dram_parameter("out", [NT, 128, H * C2], f16, isOutput=True)

    t1_loc = nc.dram_tensor("t1_loc", [NSHP, RW1], f32)
    d1_loc = nc.dram_tensor("d1_loc", [NSHP, 8], bf16)
    t2_loc = nc.dram_tensor("t2_loc", [NSHP, RW2], f32)
    d2_loc = nc.dram_tensor("d2_loc", [NSHP, 8], bf16)
    T1 = nc.dram_tensor("T1ag", [ROWS, RW1], f32, addr_space="Shared")
    T2 = nc.dram_tensor("T2ag", [ROWS, RW2], f32, addr_space="Shared")
    # one staging buffer, reused by both layers (L2 rows are narrower)
    v_dram = nc.dram_tensor("vst", [NT, 128, NBLK * RW1], f32)

    def pack_d(ap, acc, rw, d_dram, t, tagsfx):
        """acc[:, rw:rw+4] f32 -> bf16 hi/lo row, store to d_dram[t*128...]."""
        drow = ap.tile([128, 8], bf16, tag="drow" + tagsfx)
        d_hi32 = ap.tile([128, 4], f32, tag="dhi" + tagsfx)
        nc.vector.tensor_copy(drow[:, 0:4], acc[:, rw:rw + 4])
        nc.vector.tensor_copy(d_hi32[:], drow[:, 0:4])
        d_lo = ap.tile([128, 4], f32, tag="dlo" + tagsfx)
        nc.vector.tensor_tensor(out=d_lo[:], in0=acc[:, rw:rw + 4],
                                in1=d_hi32[:], op=ALU.subtract)
        nc.vector.tensor_copy(drow[:, 4:8], d_lo[:])
        nc.sync.dma_start(d_dram[ts(t, 128), :], drow[:])

    # ---------- phase A: t1 = xT.T @ W1a (For_i) ----------
    with tile.TileContext(nc) as tc:
        with (
            tc.tile_pool(name="aw", bufs=1) as wp,
            tc.tile_pool(name="aa", bufs=1) as ap,
            tc.tile_pool(name="aps", bufs=1, space="PSUM") as pp,
        ):
            w1_t = wp.tile([128, 2, RW1 + 4], f16)
            nc.sync.dma_start(
                w1_t[:], w1[:, :].rearrange("(k p) c -> p k c", p=128))
            with tc.For_i(0, NT) as t:
                xt = ap.tile([128, 2, 128], f16, tag="xt")
                nc.sync.dma_start(
                    xt[:],
                    xT[:, ds(t * 128, 128)].rearrange("(k p) c -> p k c", p=128))
                acc = pp.tile([128, RW1 + 4], f32, tag="acc")
                nc.tensor.matmul(out=acc[:], lhsT=xt[:, 0, :],
                                 rhs=w1_t[:, 0, :], start=True, stop=False)
                nc.tensor.matmul(out=acc[:], lhsT=xt[:, 1, :],
                                 rhs=w1_t[:, 1, :], start=False, stop=True)
                row = ap.tile([128, RW1], f32, tag="row")
                nc.vector.tensor_copy(row[:], acc[:, 0:RW1])
                nc.sync.dma_start(t1_loc[ts(t, 128), :], row[:])
                pack_d(ap, acc, RW1, d1_loc, t, "a")

    with nc.semaphore("cc1") as cc1:
        nc.gpsimd.collective_compute(
            "AllGather", mybir.AluOpType.bypass,
            replica_groups=[list(range(NC))],
            ins=[t1_loc[:, :].opt()], outs=[T1[:, :].opt()],
        ).then_inc(cc1, 1)
        nc.gpsimd.wait_ge(cc1, 1)

    # ---------- gather pre-pass (unrolled; indirect DMA can't be looped) ----
    def gather_pass(Tag, rw):
        with tile.TileContext(nc) as tc:
            with tc.tile_pool(name="g", bufs=2) as gp:
                for t in range(NT):
                    idx_t = gp.tile([128, NBLK], i32, tag="idx")
                    nc.sync.dma_start(idx_t[:], idxp[t, :, :])
                    vg = gp.tile([128, NBLK, rw], f32, tag="vg")
                    for b in range(NBLK):
                        nc.gpsimd.indirect_dma_start(
                            out=vg[:, b, :], out_offset=None, in_=Tag[:, :],
                            in_offset=IndirectOffsetOnAxis(
                                ap=idx_t[:, b:b + 1], axis=0))
                    nc.sync.dma_start(
                        v_dram[t, :, 0:NBLK * rw],
                        vg[:].rearrange("p b c -> p (b c)"))

    gather_pass(T1, RW1)

    # ---------- phase C: L1 message pass + fused L2 linear (For_i) ----------
    def msg_loop(tc, d_loc_t, rw, hw, hoist_cb, out_cb):
        hwq = hw // 4
        with (
            tc.tile_pool(name="mc", bufs=1) as cp,
            tc.tile_pool(name="ma", bufs=1) as ap,
            tc.tile_pool(name="mps", bufs=1, space="PSUM") as pp,
        ):
            iof = cp.tile([128, NBLK, 128], i16)
            nc.gpsimd.iota(iof[:], pattern=[[0, NBLK], [1, 128]], base=0,
                           channel_multiplier=0)
            pidx = cp.tile([128, 1], i16)
            nc.gpsimd.iota(pidx[:], pattern=[[0, 1]], base=0,
                           channel_multiplier=1)
            hoist = hoist_cb(cp)
            with tc.For_i(0, NT) as t:
                v = ap.tile([128, NBLK, rw], f32, tag="v")
                nc.sync.dma_start(
                    v[:].rearrange("p b c -> p (b c)"),
                    v_dram[ds(t, 1), :, 0:NBLK * rw])
                dtab = ap.tile([128, 8], bf16, tag="dtab")
                nc.sync.dma_start(dtab[:], d_loc_t[ts(t, 128), :])
                dlc_t = ap.tile([128, NBLK], i16, tag="dlc")
                nc.sync.dma_start(dlc_t[:], dlcp[ds(t, 1), :, :])
                m_t = ap.tile([128, NBLK, 128], bf16, tag="m")
                nc.vector.tensor_tensor(
                    out=m_t[:], in0=iof[:],
                    in1=dlc_t[:].unsqueeze(2).to_broadcast([128, NBLK, 128]),
                    op=ALU.is_equal)
                dlr_t = ap.tile([128, EB], i16, tag="dlr")
                nc.sync.dma_start(
                    dlr_t[:], dlrp[ds(t, 1), :].to_broadcast([128, EB]))
                mt_t = ap.tile([128, EB], bf16, tag="mt")
                nc.vector.tensor_tensor(
                    out=mt_t[:], in0=dlr_t[:],
                    in1=pidx[:].to_broadcast([128, EB]), op=ALU.is_equal)
                dex = pp.tile([128, NBLK * 8], f32, tag="dex")
                for b in range(NBLK):
                    nc.tensor.matmul(out=dex[:, b * 8:(b + 1) * 8],
                                     lhsT=mt_t[:, b * 128:(b + 1) * 128],
                                     rhs=dtab[:], start=True, stop=True)
                dsb = ap.tile([128, NBLK, 8], f32, tag="dsb")
                nc.vector.tensor_copy(
                    dsb[:], dex[:].rearrange("p (b k) -> p b k", k=8))
                d32 = ap.tile([128, NBLK, 4], f32, tag="d32")
                nc.vector.tensor_tensor(out=d32[:], in0=dsb[:, :, 0:4],
                                        in1=dsb[:, :, 4:8], op=ALU.add)
                e32 = ap.tile([128, NBLK, 4], f32, tag="e32")
                nc.vector.tensor_tensor(out=e32[:], in0=v[:, :, hw:hw + 4],
                                        in1=d32[:], op=ALU.add)
                e_s = ap.tile([128, NBLK, 4], f32, tag="es")
                nc.vector.tensor_scalar_mul(e_s[:], e32[:], NEG)
                nc.vector.tensor_tensor(out=e32[:], in0=e32[:], in1=e_s[:],
                                        op=ALU.max)
                g = ap.tile([128, NBLK, 4], f32, tag="g")
                nc.scalar.activation(g[:], e32[:], AF.Exp)
                wv = ap.tile([128, NBLK, hw + 4], bf16, tag="wv")
                nc.vector.tensor_tensor(
                    out=wv[:, :, 0:hw].rearrange("p b (h c) -> p b h c", h=4),
                    in0=v[:, :, 0:hw].rearrange("p b (h c) -> p b h c", h=4),
                    in1=g[:].unsqueeze(3).to_broadcast([128, NBLK, 4, hwq]),
                    op=ALU.mult)
                nc.vector.tensor_copy(wv[:, :, hw:hw + 4], g[:])
                acc = pp.tile([128, hw + 4], f32, tag="acc2")
                for b in range(NBLK):
                    nc.tensor.matmul(out=acc[:], lhsT=m_t[:, b, :],
                                     rhs=wv[:, b, :], start=(b == 0),
                                     stop=(b == NBLK - 1))
                out_cb(t, acc, ap, pp, hoist)

    def make_idn(cp):
        idn = cp.tile([128, 128], f32, tag="idn")
        iot = cp.tile([128, 1], i32, tag="iot")
        nc.gpsimd.iota(iot[:], pattern=[[0, 1]], base=0, channel_multiplier=1)
        iotf = cp.tile([128, 1], f32, tag="iotf")
        nc.vector.tensor_copy(iotf[:], iot[:])
        i2 = cp.tile([128, 128], i32, tag="i2")
        nc.gpsimd.iota(i2[:], pattern=[[1, 128]], base=0, channel_multiplier=0)
        eqi = cp.tile([128, 128], f32, tag="eqi")
        nc.vector.tensor_copy(eqi[:], i2[:])
        nc.vector.tensor_tensor(out=idn[:], in0=eqi[:],
                                in1=iotf[:].to_broadcast([128, 128]),
                                op=ALU.is_equal)
        return idn

    with tile.TileContext(nc) as tc:
        def l1_hoist(cp):
            b1_t = cp.tile([128, 256], f32, tag="b1t")
            nc.sync.dma_start(b1_t[:], b1p[0:1, :].to_broadcast([128, 256]))
            w2_t = cp.tile([128, 2, RW2 + 4], f16, tag="w2t")
            nc.sync.dma_start(
                w2_t[:], w2[:, :].rearrange("(k p) c -> p k c", p=128))
            return {"b1": b1_t, "w2": w2_t, "idn": make_idn(cp)}

        def l1_out(t, acc, ap, pp, hoist):
            rec = ap.tile([128, 4], f32, tag="rec")
            nc.vector.reciprocal(rec[:], acc[:, 256:260])
            h2 = ap.tile([128, 256], f32, tag="h2")
            nc.vector.tensor_tensor(
                out=h2[:].rearrange("p (h c) -> p h c", h=4),
                in0=acc[:, 0:256].rearrange("p (h c) -> p h c", h=4),
                in1=rec[:].unsqueeze(2).to_broadcast([128, 4, 64]),
                op=ALU.mult)
            nc.vector.tensor_tensor(out=h2[:], in0=h2[:], in1=hoist["b1"][:],
                                    op=ALU.add)
            # ELU: max(x, exp(min(x,0)) - 1)
            mn = ap.tile([128, 256], f32, tag="mn")
            nc.vector.tensor_scalar_min(mn[:], h2[:], 0.0)
            nc.scalar.activation(mn[:], mn[:], AF.Exp)
            nc.vector.tensor_scalar_add(mn[:], mn[:], -1.0)
            nc.vector.tensor_tensor(out=h2[:], in0=h2[:], in1=mn[:], op=ALU.max)
            # fused L2 linear: transpose h2 chunks, matmul with W2a
            tps = []
            for kk in range(2):
                tp = pp.tile([128, 128], f32, tag=f"tp{kk}")
                nc.tensor.transpose(out=tp[:], in_=h2[:, kk * 128:(kk + 1) * 128],
                                    identity=hoist["idn"][:])
                tk = ap.tile([128, 128], f16, tag=f"tps{kk}")
                nc.vector.tensor_copy(tk[:], tp[:])
                tps.append(tk)
            acc2 = pp.tile([128, RW2 + 4], f32, tag="acc3")
            w2_t = hoist["w2"]
            nc.tensor.matmul(out=acc2[:], lhsT=tps[0][:], rhs=w2_t[:, 0, :],
                             start=True, stop=False)
            nc.tensor.matmul(out=acc2[:], lhsT=tps[1][:], rhs=w2_t[:, 1, :],
                             start=False, stop=True)
            row = ap.tile([128, RW2], f32, tag="row2")
            nc.vector.tensor_copy(row[:], acc2[:, 0:RW2])
            nc.sync.dma_start(t2_loc[ts(t, 128), :], row[:])
            pack_d(ap, acc2, RW2, d2_loc, t, "c")

        msg_loop(tc, d1_loc, RW1, 256, l1_hoist, l1_out)

    with nc.semaphore("cc2") as cc2:
        nc.gpsimd.collective_compute(
            "AllGather", mybir.AluOpType.bypass,
            replica_groups=[list(range(NC))],
            ins=[t2_loc[:, :].opt()], outs=[T2[:, :].opt()],
        ).then_inc(cc2, 1)
        nc.gpsimd.wait_ge(cc2, 1)

    gather_pass(T2, RW2)

    # ---------- phase E: L2 message pass -> log_softmax -> out (For_i) ------
    with tile.TileContext(nc) as tc:
        def l2_hoist(cp):
            b2_t = cp.tile([128, 128], f32, tag="b2t")
            nc.sync.dma_start(b2_t[:], b2p[0:1, :].to_broadcast([128, 128]))
            return {"b2": b2_t}

        def l2_out(t, acc, ap, pp, hoist):
            rec = ap.tile([128, 4], f32, tag="rec2")
            nc.vector.reciprocal(rec[:], acc[:, 128:132])
            o = ap.tile([128, 128], f32, tag="o")
            nc.vector.tensor_tensor(
                out=o[:].rearrange("p (h c) -> p h c", h=4),
                in0=acc[:, 0:128].rearrange("p (h c) -> p h c", h=4),
                in1=rec[:].unsqueeze(2).to_broadcast([128, 4, 32]),
                op=ALU.mult)
            nc.vector.tensor_tensor(out=o[:], in0=o[:], in1=hoist["b2"][:],
                                    op=ALU.add)
            mx = ap.tile([128, 1], f32, tag="mx")
            nc.vector.reduce_max(mx[:], o[:], axis=mybir.AxisListType.X)
            nc.vector.tensor_scalar(out=o[:], in0=o[:], scalar1=mx[:, 0:1],
                                    scalar2=None, op0=ALU.subtract)
            ex = ap.tile([128, 128], f32, tag="ex")
            nc.scalar.activation(ex[:], o[:], AF.Exp)
            sm = ap.tile([128, 1], f32, tag="sm")
            nc.vector.reduce_sum(sm[:], ex[:], axis=mybir.AxisListType.X)
            nc.scalar.activation(sm[:], sm[:], AF.Ln)
            nc.vector.tensor_scalar(out=o[:], in0=o[:], scalar1=sm[:, 0:1],
                                    scalar2=None, op0=ALU.subtract)
            o16 = ap.tile([128, 128], f16, tag="o16")
            nc.vector.tensor_copy(o16[:], o[:])
            nc.sync.dma_start(outp[ds(t, 1), :, :], o16[:])

        msg_loop(tc, d2_loc, RW2, 128, l2_hoist, l2_out)

    return nc


def _split_sync_waits(nc, max_waits=1):
    import concourse.mybir as mybir
    ctr = [0]
    for f in nc.m.functions:
        for blk in f.blocks:
            new_list = []
            for ins in blk.instructions:
                si = ins.sync_info
                waits = list(si.on_wait) if si is not None and si.on_wait else []
                if len(waits) > max_waits:
                    keep = waits[:max_waits]
                    rest = waits[max_waits:]
                    for i in range(0, len(rest), max_waits):
                        ctr[0] += 1
                        nop = mybir.InstNoOp(
                            name=f"I-wsplit-{ctr[0]}", ins=[], outs=[],
                            engine=ins.engine, bass_nofuse=True)
                        nop.sync_info = mybir.SyncInfo(
                            on_wait=rest[i:i + max_waits], on_update=[])
                        new_list.append(nop)
                    ins.sync_info = mybir.SyncInfo(
                        on_wait=keep,
                        on_update=list(si.on_update) if si.on_update else [])
                new_list.append(ins)
            blk.instructions[:] = new_list


# input order must match ExternalInput declaration order in _build_nc
_IN_NAMES = ["xT", "w1", "w2", "idx", "dlc", "dlr", "b1r", "b2r"]
_CACHE = {}
_LOCK = threading.Lock()


def _compile():
    """Build the Bass module and compile the sharded executable. Cached."""
    with _LOCK:
        if "compiled" in _CACHE:
            return _CACHE["compiled"]
        import os
        import jax
        import jax.numpy as jnp
        from jax.sharding import Mesh, PartitionSpec, NamedSharding
        try:
            jax.config.update("jax_compilation_cache_dir", "/tmp/jax_bass_cache")
            jax.config.update("jax_persistent_cache_min_entry_size_bytes", -1)
            jax.config.update("jax_persistent_cache_min_compile_time_secs", 0.0)
        except Exception:
            pass
        import warnings
        with warnings.catch_warnings():
            warnings.simplefilter("ignore")
            from jax.experimental.shard_map import shard_map
        import concourse.bass2jax as b2j
        from concourse import mybir

        import time as _t
        import hashlib
        import inspect
        import pickle
        import zstandard

        key_src = inspect.getsource(_build_nc) + inspect.getsource(
            _split_sync_waits) + "v1"
        key = hashlib.sha256(key_src.encode()).hexdigest()[:16]
        bir_path = f"/tmp/bass_gat_bir_{key}.pkl"
        _ts = _t.time()
        meta = None
        try:
            with open(bir_path, "rb") as fh:
                meta = pickle.load(fh)
        except Exception:
            meta = None
        if meta is None:
            nc_full = _build_nc()
            _split_sync_waits(nc_full, 1)
            partition_name = (nc_full.partition_id_tensor.name
                              if nc_full.partition_id_tensor else None)
            in_specs_m, out_specs_m = [], []
            for alloc in nc_full.m.functions[0].allocations:
                if not isinstance(alloc, mybir.MemoryLocationSet):
                    continue
                name = alloc.memorylocations[0].name
                if alloc.kind == "ExternalInput":
                    if name != partition_name:
                        in_specs_m.append(
                            (name, tuple(alloc.tensor_shape),
                             np.dtype(mybir.dt.np(alloc.dtype)).str))
                elif alloc.kind == "ExternalOutput":
                    out_specs_m.append(
                        (name, tuple(alloc.tensor_shape),
                         np.dtype(mybir.dt.np(alloc.dtype)).str))
            meta = {
                "bir_zst": zstandard.ZstdCompressor().compress(
                    nc_full.to_json_bytes()),
                "arch": nc_full.m.arch,
                "has_collectives": nc_full.has_collectives,
                "partition_name": partition_name,
                "in_specs": in_specs_m,
                "out_specs": out_specs_m,
            }
            try:
                tmp = bir_path + ".tmp"
                with open(tmp, "wb") as fh:
                    pickle.dump(meta, fh)
                os.replace(tmp, bir_path)
            except Exception:
                pass

        class _M:
            pass

        class _NcShim:
            def __init__(self, bir, arch, has_cc):
                self._bir = bir
                self.m = _M()
                self.m.arch = arch
                self.has_collectives = has_cc
                self.target_bir_lowering = False
                self.dbg_addr = None

            def to_json_bytes(self):
                return self._bir

        nc = _NcShim(
            zstandard.ZstdDecompressor().decompress(meta["bir_zst"]),
            meta["arch"], meta["has_collectives"])
        _CACHE["t_build"] = _t.time() - _ts
        b2j.install_neuronx_cc_hook()

        partition_name = meta["partition_name"]
        in_names = [s[0] for s in meta["in_specs"]]
        out_names, out_avals, out_shapes = [], [], []
        for name, shape, dts in meta["out_specs"]:
            dtype = np.dtype(dts).type
            out_names.append(name)
            out_avals.append(jax.core.ShapedArray(shape, dtype))
            out_shapes.append((shape, dtype))
        n_params = len(in_names)
        n_outs = len(out_avals)
        all_names = in_names + out_names
        if partition_name is not None:
            all_names = all_names + [partition_name]
        donate = tuple(range(n_params, n_params + n_outs))

        def _body(*args):
            operands = list(args)
            if partition_name is not None:
                operands.append(b2j.partition_id_tensor())
            outs = b2j._bass_exec_p.bind(
                *operands, out_avals=tuple(out_avals),
                in_names=tuple(all_names), out_names=tuple(out_names),
                lowering_input_output_aliases=(),
                sim_require_finite=True, sim_require_nnan=True, nc=nc)
            return tuple(outs)

        devices = jax.devices()[:NC]
        mesh = Mesh(np.asarray(devices), ("core",))
        sharding = NamedSharding(mesh, PartitionSpec("core"))
        in_specs = (PartitionSpec("core"),) * (n_params + n_outs)
        out_specs = (PartitionSpec("core"),) * n_outs
        jitted = jax.jit(
            shard_map(_body, mesh=mesh, in_specs=in_specs,
                      out_specs=out_specs, check_rep=False),
            donate_argnums=donate, keep_unused=True)

        in_structs = [
            jax.ShapeDtypeStruct((NC * shp[0],) + tuple(shp[1:]),
                                 np.dtype(dts).type, sharding=sharding)
            for (_, shp, dts) in meta["in_specs"]]
        zero_structs = [jax.ShapeDtypeStruct((NC * s[0],) + tuple(s[1:]), dt,
                                             sharding=sharding)
                        for (s, dt) in out_shapes]
        _ts = _t.time()
        def _do_compile():
            lowered = jitted.lower(*in_structs, *zero_structs)
            _CACHE["t_lower"] = _t.time() - _ts
            return lowered.compile()
        try:
            compiled = b2j.fast_dispatch_compile(_do_compile)
        except Exception:
            compiled = jitted.lower(*in_structs, *zero_structs).compile()
        _CACHE["t_compile"] = _t.time() - _ts

        zfuns = [jax.jit(lambda s=s, dt=dt: jnp.zeros((NC * s[0],) + tuple(s[1:]), dt),
                         out_shardings=sharding) for (s, dt) in out_shapes]
        # pre-create the first set of donated output buffers on-device now,
        # off the critical path
        _CACHE["zeros"] = [zf() for zf in zfuns]
        _CACHE["compiled"] = (compiled, in_names, out_names, out_shapes,
                              sharding, zfuns)
        return _CACHE["compiled"]


def _warmup_transfer():
    """Tiny transfer issued ASAP: the terminal's first-transfer-after-teardown
    stall (if any) burns here, overlapped with compile + host prep."""
    try:
        import jax
        from jax.sharding import Mesh, PartitionSpec, NamedSharding
        devices = jax.devices()[:NC]
        mesh = Mesh(np.asarray(devices), ("core",))
        sh = NamedSharding(mesh, PartitionSpec("core"))
        d = jax.device_put(np.zeros((NC, 8), np.float32), sh)
        jax.block_until_ready(d)
    except Exception:
        pass


def _start_precompile():
    if "thread" in _CACHE:
        return _CACHE["thread"]
    wt = threading.Thread(target=_warmup_transfer, daemon=True)
    _CACHE["warmup"] = wt
    wt.start()
    th = threading.Thread(target=lambda: _try_compile(), daemon=True)
    _CACHE["thread"] = th
    th.start()
    return th


def _try_compile():
    try:
        _compile()
    except Exception as e:  # surface later in kernel()
        _CACHE["compile_error"] = e


def kernel(**inputs):
    import os
    import time as _time
    dbg = os.environ.get("KERNEL_DEBUG_TIMING")
    t_all = _time.time()
    th = _start_precompile()

    from concurrent.futures import ThreadPoolExecutor
    if any(not isinstance(v, np.ndarray) for v in inputs.values()):
        # jax-array inputs: fetch to host in parallel (d2h RPCs overlap)
        with ThreadPoolExecutor(max_workers=8) as pool:
            futs = {k: pool.submit(np.asarray, v) for k, v in inputs.items()}
            inputs = {k: f.result() for k, f in futs.items()}
    # transform x first and start its (big) transfer while the edge
    # bucketing runs on the CPU
    xs = _prep_x(inputs["x"])
    import jax
    from jax.sharding import Mesh, PartitionSpec, NamedSharding
    devices = jax.devices()[:NC]
    mesh = Mesh(np.asarray(devices), ("core",))
    sharding = NamedSharding(mesh, PartitionSpec("core"))
    dev_map = {"xT": jax.device_put(xs.reshape(NC * F_IN, NSHP), sharding)}
    if dbg:
        print(f"[k] xT dispatched {_time.time()-t_all:.2f}s", flush=True)

    W1a, W2a, idx_t, dlc, dlr, b1r, b2r = _prep_rest(
        inputs["edge_index"], inputs["W1"], inputs["att_src1"],
        inputs["att_dst1"], inputs["b1"], inputs["W2"], inputs["att_src2"],
        inputs["att_dst2"], inputs["b2"])
    if dbg:
        print(f"[k] host_prep {_time.time()-t_all:.2f}s", flush=True)

    t0 = _time.time()
    per_core = {
        "w1": np.broadcast_to(W1a[None], (NC,) + W1a.shape),
        "w2": np.broadcast_to(W2a[None], (NC,) + W2a.shape),
        "idx": idx_t, "dlc": dlc, "dlr": dlr,
        "b1r": np.broadcast_to(b1r[None], (NC,) + b1r.shape),
        "b2r": np.broadcast_to(b2r[None], (NC,) + b2r.shape),
    }
    # flatten [NC, s0, ...] -> [NC*s0, ...] for shard_map axis-0 sharding
    flat = {k: np.ascontiguousarray(v).reshape((-1,) + v.shape[2:])
            for k, v in per_core.items()}

    # ship the rest while the compile thread works
    for name in _IN_NAMES:
        if name not in dev_map:
            dev_map[name] = jax.device_put(flat[name], sharding)
    if dbg:
        print(f"[k] device_put dispatch {_time.time()-t0:.2f}s", flush=True)

    th.join()
    if "compile_error" in _CACHE:
        raise _CACHE["compile_error"]
    compiled, in_names, out_names, out_shapes, _, zfuns = _CACHE["compiled"]
    if dbg:
        print(f"[k] join {_time.time()-t0:.2f}s (build {_CACHE.get('t_build',0):.2f}"
              f" lower {_CACHE.get('t_lower',0):.2f}"
              f" compile {_CACHE.get('t_compile',0):.2f})", flush=True)
    dev_in = [dev_map[name] for name in in_names]
    zeros = _CACHE.pop("zeros", None) or [zf() for zf in zfuns]
    jax.block_until_ready(dev_in)
    if dbg:
        print(f"[k] inputs ready {_time.time()-t0:.2f}s", flush=True)

    out_arrs = compiled(*dev_in, *zeros)
    jax.block_until_ready(out_arrs)
    if dbg:
        print(f"[k] exec {_time.time()-t0:.2f}s", flush=True)

    oi = out_names.index("out")
    shards = out_arrs[oi].addressable_shards
    full = np.empty((NC * NT, 128, H * C2), np.float16)
    with ThreadPoolExecutor(max_workers=8) as pool:
        def _pull(s):
            full[s.index[0]] = np.asarray(s.data)
        list(pool.map(_pull, shards))
    full = full.reshape(NC, NSHP, H * C2)
    res = full[:, :NSH, :].reshape(N, H * C2).astype(np.float32)
    if dbg:
        print(f"[k] fetch {_time.time()-t0:.2f}s", flush=True)
    kernel.last_wall_s = _time.time() - t0
    kernel.total_wall_s = _time.time() - t_all
    return res


def _graceful_shutdown():
    """Release device state before exit: an abrupt client disconnect leaves
    the axon terminal cleaning up lazily, which stalls the next process's
    first transfer by tens of seconds."""
    try:
        import sys
        if "jax" not in sys.modules:
            return
        import gc
        import jax
        _CACHE.clear()
        gc.collect()
        try:
            jax.clear_caches()
        except Exception:
            pass
        try:
            import jax.extend.backend as jeb
            jeb.clear_backends()
        except Exception:
            pass
        gc.collect()
    except Exception:
        pass


import atexit  # noqa: E402
atexit.register(_graceful_shutdown)

_start_precompile()
